# revision 1
# baseline (speedup 1.0000x reference)
"""AdaptiveCoverageAttention TRN2 kernel: 8-way (batch x head-group) sharded.

Sharding: core c in 0..7 -> batch b = c//4, head-group hg = c%4 (4 heads each).
Each core computes its 4 heads' attention + its partial output projection;
the host sums the 4 partials per batch (and adds b_out). No collectives.

v3: the attention exp stream (ScalarE-bound, ~1.1us per [128,1024] exp tile)
runs nearly the whole kernel; everything else hides inside its PE slack.
- Prefix: dual-ring DMA (sync+scalar DGE), K^T for pair 0, Q^T for (p0,ic0),
  first two V tiles, and the gate/coverage MLPs (pooled sums reduced per
  DMA chunk so they pipeline with the loads).
- Stream: per j-tile, S^T (K=64 row slices), exp with per-partition coverage
  bias, AV with M=64 V tiles, and softmax denominators as concurrent M=1
  col-tiles into one PSUM bank (partitions 0/32/64/96). One "job" (V tile,
  K-p1/Q chunk, or ic0 out-proj half) is emitted per j-tile to fill slack.
- PSUM: S 2x[128,1024] (4 banks) + AV pair accumulator (2) + denominators
  (1) + job scratch (1) = 8 banks exactly.
- Tail: remaining out-projection with a wide PSUM pool, bf16 output DMA.
"""
import sys

sys.path.insert(0, "/opt/trn_rl_repo")

import numpy as np

B, N, D, H = 2, 2048, 1024, 16
HD = D // H            # 64
HPC = 4                # heads per core
NCORES = 8
IC_W = 1024

_COMPILED = {}


def _bf16(x):
    import ml_dtypes
    return np.ascontiguousarray(np.asarray(x, np.float32)).astype(ml_dtypes.bfloat16)


def build(n=N):
    """Build the per-core Bass graph for sequence length n (n % 1024 == 0)."""
    import contextlib

    import concourse.bacc as bacc
    import concourse.tile as tile
    from concourse import mybir

    f32 = mybir.dt.float32
    bf16 = mybir.dt.bfloat16
    AFT = mybir.ActivationFunctionType

    NJ = n // 128          # 16 j-tiles (key positions)
    NI = n // 512          # 4 512-wide chunks (projection phase)
    NI2 = n // IC_W        # 2 i-chunks (query positions, attention phase)
    DC = D // 128          # 8 contraction chunks

    nc = bacc.Bacc("TRN2", target_bir_lowering=False, debug=False,
                   num_devices=NCORES)

    dram = lambda name, shape, dt, kind: nc.dram_tensor(name, shape, dt, kind=kind).ap()
    XT = dram("xT", [D, n], bf16, "ExternalInput")
    WQK = dram("wqk", [D, 512], bf16, "ExternalInput")
    WV = dram("wv", [D, 256], bf16, "ExternalInput")
    WO = dram("wo", [256, D], bf16, "ExternalInput")
    COVT = dram("covT", [1, n], bf16, "ExternalInput")
    WCE1 = dram("wce1", [1, 256], bf16, "ExternalInput")
    BCE1 = dram("bce1", [128, 2], f32, "ExternalInput")
    WCE2 = dram("wce2", [128, 8], bf16, "ExternalInput")
    BCE2 = dram("bce2", [128, 4], f32, "ExternalInput")
    WFG1 = dram("wfg1", [D, 256], f32, "ExternalInput")
    BFG1 = dram("bfg1", [128, 2], f32, "ExternalInput")
    WFG2 = dram("wfg2", [128, 2], f32, "ExternalInput")
    BFG2 = dram("bfg2", [1, 1], f32, "ExternalInput")
    OUT = dram("out", [n, D], bf16, "ExternalOutput")

    with tile.TileContext(nc) as tc, contextlib.ExitStack() as ctx:
        consts = ctx.enter_context(tc.tile_pool(name="consts", bufs=1))
        xtp = ctx.enter_context(tc.tile_pool(name="xtp", bufs=DC))
        qkv = ctx.enter_context(tc.tile_pool(name="qkv", bufs=1))
        big2 = ctx.enter_context(tc.tile_pool(name="big2", bufs=1))
        ep = ctx.enter_context(tc.tile_pool(name="ep", bufs=6))
        rp = ctx.enter_context(tc.tile_pool(name="rp", bufs=3))
        wfp = ctx.enter_context(tc.tile_pool(name="wfp", bufs=8))
        yp = ctx.enter_context(tc.tile_pool(name="yp", bufs=4))

        # ---- constants into SBUF (split across both DGE rings) ----
        wqk_sb = consts.tile([128, DC, 512], bf16)
        wv_sb = consts.tile([128, DC, 256], bf16)
        wo_sb = consts.tile([128, 2, D], bf16)
        covT_sb = consts.tile([1, n], bf16)
        wce1_sb = consts.tile([1, 256], bf16)
        bce1_sb = consts.tile([128, 2], f32)
        wce2_sb = consts.tile([128, 8], bf16)
        bce2_sb = consts.tile([128, 4], f32)
        bfg1_sb = consts.tile([128, 2], f32)
        wfg2_sb = consts.tile([128, 2], f32)
        bfg2_sb = consts.tile([1, 1], f32)
        for dc in range(DC):
            nc.sync.dma_start(out=wqk_sb[:, dc, :], in_=WQK[dc * 128:(dc + 1) * 128, :])
            nc.scalar.dma_start(out=wv_sb[:, dc, :], in_=WV[dc * 128:(dc + 1) * 128, :])
        for pt in range(2):
            nc.scalar.dma_start(out=wo_sb[:, pt, :], in_=WO[pt * 128:(pt + 1) * 128, :])
        nc.scalar.dma_start(out=covT_sb, in_=COVT)
        nc.scalar.dma_start(out=wce1_sb, in_=WCE1)
        nc.scalar.dma_start(out=bce1_sb, in_=BCE1)
        nc.scalar.dma_start(out=wce2_sb, in_=WCE2)
        nc.scalar.dma_start(out=bce2_sb, in_=BCE2)
        nc.scalar.dma_start(out=bfg1_sb, in_=BFG1)
        nc.scalar.dma_start(out=wfg2_sb, in_=WFG2)
        nc.scalar.dma_start(out=bfg2_sb, in_=BFG2)

        ones_f = consts.tile([1, 128], f32)
        nc.vector.memset(ones_f, 1.0)
        ones_bf = consts.tile([128, 1], bf16)
        nc.vector.memset(ones_bf, 1.0)

        pooled4 = consts.tile([128, DC, NI], f32)
        pooled_sb = consts.tile([128, DC], f32)
        hidg_sb = consts.tile([128, 2], f32)
        g_sb = consts.tile([1, 1], f32)
        gb_sb = consts.tile([128, 1], f32)
        bias_sb = consts.tile([128, NJ, 4], f32)

        # ---- xT DMA: j-chunk-major, alternating DGE rings; pooled partial
        #      sums reduced per chunk so they pipeline with the loads ----
        xts = []
        for dc in range(DC):
            xt = xtp.tile([128, NI, 512], bf16, tag="xt", name=f"xt{dc}")
            xts.append(xt)
        for jc in range(NI):
            for dc in range(DC):
                eng = nc.sync if dc % 2 == 0 else nc.scalar
                eng.dma_start(out=xts[dc][:, jc, :],
                              in_=XT[dc * 128:(dc + 1) * 128,
                                     jc * 512:(jc + 1) * 512])
            for dc in range(DC):
                nc.vector.reduce_sum(pooled4[:, dc, jc:jc + 1], xts[dc][:, jc, :],
                                     axis=mybir.AxisListType.X)
        for dc in range(DC):
            nc.vector.reduce_sum(pooled_sb[:, dc:dc + 1], pooled4[:, dc, :],
                                 axis=mybir.AxisListType.X)

        qt_sb = qkv.tile([128, 2, n], bf16)
        ktp_sb = qkv.tile([128, 2, n], bf16)
        vaug_sb = qkv.tile([128, NJ, 4, 65], bf16)
        nc.vector.memset(vaug_sb, 1.0)

        # ---- prefix: K^T(p0), Q^T(p0, ic0), V(0..1), MLPs ----
        with tc.tile_pool(name="pfA", bufs=3, space="PSUM") as pfA, \
             tc.tile_pool(name="pft", bufs=1, space="PSUM") as pft:

            def qk_chunk(pool, cb, ic, tag="qk", bufs=2):
                pq = pool.tile([128, 512], f32, tag=tag, name=f"pq{cb}_{ic}",
                               bufs=bufs)
                for dc in range(DC):
                    nc.tensor.matmul(pq, wqk_sb[:, dc, cb * 128:(cb + 1) * 128],
                                     xts[dc][:, ic, :],
                                     start=(dc == 0), stop=(dc == DC - 1))
                dst = (ktp_sb[:, cb - 2, ic * 512:(ic + 1) * 512] if cb >= 2
                       else qt_sb[:, cb, ic * 512:(ic + 1) * 512])
                nc.vector.tensor_copy(dst, pq)

            def v_chunk(pool, it, tag="v", bufs=1):
                pv = pool.tile([128, 4, 64], f32, tag=tag, name=f"pv{it}",
                               bufs=bufs)
                for dc in range(DC):
                    nc.tensor.matmul(pv, xts[dc][:, it // 4, (it % 4) * 128:
                                                 (it % 4) * 128 + 128],
                                     wv_sb[:, dc, :],
                                     start=(dc == 0), stop=(dc == DC - 1))
                nc.vector.tensor_copy(vaug_sb[:, it, :, 0:64], pv)

            for ic in range(NI):
                qk_chunk(pfA, 2, ic)
            # gate MLP (tiny, plain f32 matmuls); wfg1 streamed per d-chunk
            wfs = []
            for dc in range(DC):
                wf = wfp.tile([128, 256], f32, tag="wfg1", name=f"wf{dc}")
                nc.scalar.dma_start(out=wf, in_=WFG1[dc * 128:(dc + 1) * 128, :])
                wfs.append(wf)
            pg = pft.tile([128, 512], f32, tag="tiny", name="pg")
            for mc in range(2):
                for dc in range(DC):
                    nc.tensor.matmul(pg[:, mc:mc + 1],
                                     wfs[dc][:, mc * 128:(mc + 1) * 128],
                                     pooled_sb[:, dc:dc + 1],
                                     start=(dc == 0), stop=(dc == DC - 1))
            for mc in range(2):
                nc.scalar.activation(out=hidg_sb[:, mc:mc + 1], in_=pg[:, mc:mc + 1],
                                     func=AFT.Silu, bias=bfg1_sb[:, mc:mc + 1],
                                     scale=1.0 / n)
            pgp = pft.tile([128, 512], f32, tag="tiny")
            for mc in range(2):
                nc.tensor.matmul(pgp[0:1, 0:1], hidg_sb[:, mc:mc + 1],
                                 wfg2_sb[:, mc:mc + 1],
                                 start=(mc == 0), stop=(mc == 1))
            nc.scalar.activation(out=g_sb, in_=pgp[0:1, 0:1], func=AFT.Sigmoid,
                                 bias=bfg2_sb, scale=1.0)
            pgb = pft.tile([128, 512], f32, tag="tiny")
            nc.tensor.matmul(pgb[:, 0:1], ones_f, g_sb, start=True, stop=True)
            nc.vector.tensor_copy(gb_sb, pgb[:, 0:1])

            # coverage MLP (tiny, plain f32): hidden^T then cov (scaled by g)
            hidc_sb = big2.tile([128, 2, n], bf16, tag="big", name="hidc")
            for mc in range(2):
                for jc in range(NI):
                    ph = pft.tile([128, 512], f32, tag="tiny")
                    nc.tensor.matmul(ph, wce1_sb[:, mc * 128:(mc + 1) * 128],
                                     covT_sb[:, jc * 512:(jc + 1) * 512],
                                     start=True, stop=True)
                    nc.scalar.activation(out=hidc_sb[:, mc, jc * 512:(jc + 1) * 512],
                                         in_=ph, func=AFT.Silu,
                                         bias=bce1_sb[:, mc:mc + 1], scale=1.0)
            for jt in range(NJ):
                pc = pft.tile([128, 512], f32, tag="tiny")
                for mc in range(2):
                    nc.tensor.matmul(pc[:, 0:4], hidc_sb[:, mc, jt * 128:(jt + 1) * 128],
                                     wce2_sb[:, mc * 4:(mc + 1) * 4],
                                     start=(mc == 0), stop=(mc == 1))
                nc.vector.tensor_add(bias_sb[:, jt, :], pc[:, 0:4], bce2_sb)
            for jt in range(NJ):
                nc.vector.tensor_scalar_mul(out=bias_sb[:, jt, :],
                                            in0=bias_sb[:, jt, :], scalar1=gb_sb)
            # exp table warmup so the first stream exp pays no table load
            warm_sb = consts.tile([1, 128], bf16)
            nc.scalar.activation(out=warm_sb, in_=pgb[0:1, 0:128], func=AFT.Exp,
                                 scale=0.001)

            for ic in range(NI):
                qk_chunk(pfA, 0, ic)
            for it in range(NJ):
                v_chunk(pfA, it, bufs=2)
            for ic in range(NI):
                qk_chunk(pfA, 3, ic)
            for ic in range(NI):
                qk_chunk(pfA, 1, ic)


        # ---- attention: per head pair, S^T -> exp -> [V|1]^T P^T ----
        scale = float(HD) ** -0.5
        attn_sb = big2.tile([128, 2, n], bf16, tag="big", name="attn")
        with tc.tile_pool(name="pop", bufs=2, space="PSUM") as pop, \
             tc.tile_pool(name="pss", bufs=2, space="PSUM") as pss:
            for p in range(2):
                for ic in range(NI2):
                    po = [pop.tile([65, IC_W], f32, tag="o",
                                   name=f"po{p}_{ic}_{i}") for i in range(2)]
                    for jt in range(NJ):
                        js = slice(jt * 128, (jt + 1) * 128)
                        pss_t, es = [], []
                        for hh in range(2):
                            lo = hh * 64
                            ps_ = pss.tile([128, IC_W], f32, tag="s",
                                           name=f"s{p}_{ic}_{jt}_{hh}")
                            for q in range(IC_W // 512):
                                nc.tensor.matmul(
                                    ps_[:, q * 512:(q + 1) * 512],
                                    ktp_sb[lo:lo + 64, p, js],
                                    qt_sb[lo:lo + 64, p,
                                          ic * IC_W + q * 512:
                                          ic * IC_W + (q + 1) * 512],
                                    start=True, stop=True)
                            pss_t.append(ps_)
                        for hh in range(2):
                            h = 2 * p + hh
                            e = ep.tile([128, IC_W], bf16, tag="e",
                                        name=f"e{p}_{ic}_{jt}_{hh}")
                            nc.scalar.activation(out=e, in_=pss_t[hh],
                                                 func=AFT.Exp,
                                                 bias=bias_sb[:, jt, h:h + 1],
                                                 scale=scale)
                            es.append(e)
                        st, sp = (jt == 0), (jt == NJ - 1)
                        for hh in range(2):
                            h = 2 * p + hh
                            for q in range(IC_W // 512):
                                nc.tensor.matmul(
                                    po[hh][:, q * 512:(q + 1) * 512],
                                    vaug_sb[:, jt, h, :],
                                    es[hh][:, q * 512:(q + 1) * 512],
                                    start=st, stop=sp)
                    # normalize: O^T rows 0..63, denominator row 64
                    osl = slice(ic * IC_W, (ic + 1) * IC_W)
                    for hh in range(2):
                        lo = hh * 64
                        dd = rp.tile([1, IC_W], f32, tag="dd",
                                     name=f"dd{p}_{ic}_{hh}")
                        nc.vector.tensor_copy(dd, po[hh][64:65, :])
                        rr = rp.tile([1, IC_W], f32, tag="rr",
                                     name=f"rr{p}_{ic}_{hh}")
                        nc.vector.reciprocal_approx_fast(out=rr, in_=dd)
                        recb = rp.tile([64, IC_W], f32, tag="recb",
                                       name=f"recb{p}_{ic}_{hh}")
                        nc.gpsimd.partition_broadcast(recb, rr)
                        nc.vector.tensor_mul(attn_sb[lo:lo + 64, p, osl],
                                             po[hh][0:64, :], recb)
        done_its = 0

        # ---- tail: remaining out-projection with a wide PSUM pool ----
        with tc.tile_pool(name="psy", bufs=2, space="PSUM") as psy:
            for it in range(done_its, NJ):
                py = psy.tile([128, D], f32, tag="y")
                for pt in range(2):
                    for half in range(2):
                        nc.tensor.matmul(
                            py[:, half * 512:(half + 1) * 512],
                            attn_sb[:, pt, it * 128:(it + 1) * 128],
                            wo_sb[:, pt, half * 512:(half + 1) * 512],
                            start=(pt == 0), stop=(pt == 1))
                y_sb = yp.tile([128, D], bf16, tag="y_sb", name=f"ysb{it}")
                if it % 2 == 0:
                    nc.vector.tensor_copy(y_sb, py)
                else:
                    nc.scalar.copy(y_sb, py)
                nc.sync.dma_start(out=OUT[it * 128:(it + 1) * 128, :], in_=y_sb)

    nc.compile()
    return nc


def make_in_maps(x, coverage, w_qkv, w_out, b_out, w_ce1, b_ce1, w_ce2, b_ce2,
                 w_fg1, b_fg1, w_fg2, b_fg2, n=N):
    f = np.float32
    x = np.asarray(x, f)
    coverage = np.asarray(coverage, f)
    w_qkv = np.asarray(w_qkv, f)
    w_out = np.asarray(w_out, f)
    in_maps = []
    for c in range(NCORES):
        b, hg = divmod(c, 4)
        cs, ce = hg * 256, (hg + 1) * 256
        wq = w_qkv[:, 0 * D + cs:0 * D + ce]
        wk = w_qkv[:, 1 * D + cs:1 * D + ce]
        wv = w_qkv[:, 2 * D + cs:2 * D + ce]
        m = {
            "xT": _bf16(x[b].T),
            "wqk": _bf16(np.concatenate([wq, wk], axis=1)),
            "wv": _bf16(wv),
            "wo": _bf16(w_out[cs:ce, :]),
            "covT": _bf16(coverage[b, :, 0][None, :]),
            "wce1": _bf16(w_ce1),
            "bce1": np.ascontiguousarray(np.asarray(b_ce1, f).reshape(2, 128).T),
            "wce2": _bf16(
                np.asarray(w_ce2, f)[:, 4 * hg:4 * hg + 4].reshape(2, 128, 4)
                .transpose(1, 0, 2).reshape(128, 8)),
            "bce2": np.tile(np.asarray(b_ce2, f)[4 * hg:4 * hg + 4][None, :], (128, 1)),
            "wfg1": np.ascontiguousarray(np.asarray(w_fg1, f)),
            "bfg1": np.ascontiguousarray(np.asarray(b_fg1, f).reshape(2, 128).T),
            "wfg2": np.ascontiguousarray(np.asarray(w_fg2, f).reshape(2, 128).T),
            "bfg2": np.asarray(b_fg2, f).reshape(1, 1),
        }
        in_maps.append(m)
    return in_maps


def kernel(**inputs):
    from concourse.bass_utils import run_bass_kernel_spmd
    if "nc" not in _COMPILED:
        _COMPILED["nc"] = build(N)
    nc = _COMPILED["nc"]
    in_maps = make_in_maps(**inputs)
    res = run_bass_kernel_spmd(nc, in_maps, core_ids=list(range(NCORES)))
    outs = [np.asarray(res.results[c]["out"], dtype=np.float32)
            for c in range(NCORES)]
    b_out = np.asarray(inputs["b_out"], np.float32)
    full = np.stack([
        outs[0] + outs[1] + outs[2] + outs[3] + b_out[None, :],
        outs[4] + outs[5] + outs[6] + outs[7] + b_out[None, :],
    ]).astype(np.float32)
    return full



# revision 10
# speedup vs baseline: 1.1137x; 1.1137x over previous
"""AdaptiveCoverageAttention TRN2 kernel: 8-way (batch x head-group) sharded.

Sharding: core c in 0..7 -> batch b = c//4, head-group hg = c%4 (4 heads each).
Each core computes its 4 heads' attention + its partial output projection;
the host sums the 4 partials per batch (and adds b_out). No collectives.

v4: PE-roofline oriented rewrite (PE ~393k cycles/core @2.4GHz = 164us).
- Host pre-packs all weights/x partition-major so the whole input loads in
  ~15 large DMAs split across the two HW DGE rings (sync+scalar).
- Prefix: cov MLP first (fills PE while x streams in), then K0/Q0 chunks,
  V tiles, K1/Q1, gate MLP. PSUM->SBUF copies spread over vector/scalar/
  gpsimd.
- Stream (per pair p, i-chunk ic): software-pipelined jt loop; emission
  order exp(jt) -> S(jt+1) -> AV(jt) keeps the in-order PE queue busy
  (S(jt+1) runs while exp(jt) is on ScalarE) so the PE p-state stays high.
- ~31% of exp tiles run on the idle VectorE via a Schraudolph bf16
  exp approximation (int16 convert + bitcast, mean-centered C=-7.37;
  adds ~7e-3 rel err, well under the 2e-2 budget) so ScalarE (1.1us per
  [128,1024] exp tile) stops being the stream bottleneck.
- Normalize: reciprocal on DVE straight from PSUM, partition_broadcast on
  GpSimd, final mul on DVE.
- Tail: out-projection per 128-row tile, copies alternate scalar/vector,
  DMA on sync.
"""
import sys

sys.path.insert(0, "/opt/trn_rl_repo")

import numpy as np

B, N, D, H = 2, 2048, 1024, 16
HD = D // H            # 64
HPC = 4                # heads per core
NCORES = 8
IC_W = 1024

_COMPILED = {}

SCHRAUD_A = float(128.0 * np.log2(np.e))
SCHRAUD_B = float(127.0 * 128.0 - 7.37)


def _bf16(x):
    import ml_dtypes
    return np.ascontiguousarray(np.asarray(x, np.float32)).astype(ml_dtypes.bfloat16)


import os as _os
_DVE_OFF = bool(int(_os.environ.get("KDVE_OFF", "0")))


def _dve_tile(jt, hh):
    """Which exp tiles run on VectorE (Schraudolph). ~31% of tiles."""
    if _DVE_OFF:
        return False
    return (hh == 1 and jt % 2 == 1 and jt >= 3) or (hh == 0 and jt in (6, 10, 14))


def build(n=N):
    """Build the per-core Bass graph for sequence length n (n % 1024 == 0)."""
    import contextlib

    import concourse.bacc as bacc
    import concourse.tile as tile
    from concourse import mybir

    f32 = mybir.dt.float32
    bf16 = mybir.dt.bfloat16
    i16 = mybir.dt.int16
    AFT = mybir.ActivationFunctionType
    ALU = mybir.AluOpType

    NJ = n // 128          # 16 j-tiles (key positions)
    NI = n // 512          # 4 512-wide chunks (projection phase)
    NI2 = n // IC_W        # 2 i-chunks (query positions, attention phase)
    DC = D // 128          # 8 contraction chunks

    nc = bacc.Bacc("TRN2", target_bir_lowering=False, debug=False,
                   num_devices=NCORES)

    dram = lambda name, shape, dt, kind: nc.dram_tensor(name, shape, dt, kind=kind).ap()
    # host pre-packed partition-major layouts
    XT = dram("xT", [128, 2, DC, 1024], bf16, "ExternalInput")     # (p, jc2, dc, tok)
    WQK = dram("wqk", [128, 4, DC, 128], bf16, "ExternalInput")    # (p, cb, dc, col)
    WV = dram("wv", [128, DC, 256], bf16, "ExternalInput")
    WO = dram("wo", [128, 2, D], bf16, "ExternalInput")
    COVT = dram("covT", [1, n], bf16, "ExternalInput")
    WCE1 = dram("wce1", [1, 256], bf16, "ExternalInput")
    BCE1 = dram("bce1", [128, 2], f32, "ExternalInput")
    WCE2 = dram("wce2", [128, 8], bf16, "ExternalInput")
    BCE2 = dram("bce2", [128, 4], f32, "ExternalInput")
    WFG1 = dram("wfg1", [128, DC, 256], bf16, "ExternalInput")
    BFG1 = dram("bfg1", [128, 2], f32, "ExternalInput")
    WFG2 = dram("wfg2", [128, 2], f32, "ExternalInput")
    BFG2 = dram("bfg2", [1, 1], f32, "ExternalInput")
    OUT = dram("out", [n, D], bf16, "ExternalOutput")
    dbg = bool(int(__import__("os").environ.get("KDBG", "0")))
    if dbg:
        DBG_Q = dram("dbg_q", [128, 2, n], bf16, "ExternalOutput")
        DBG_K = dram("dbg_k", [128, 2, n], bf16, "ExternalOutput")
        DBG_V = dram("dbg_v", [128, NJ, 4, 65], bf16, "ExternalOutput")
        DBG_B = dram("dbg_b", [128, NJ, 4], f32, "ExternalOutput")
        DBG_A = dram("dbg_a", [128, 2, n], bf16, "ExternalOutput")

    with tile.TileContext(nc) as tc, contextlib.ExitStack() as ctx:
        consts = ctx.enter_context(tc.tile_pool(name="consts", bufs=1))
        xtp = ctx.enter_context(tc.tile_pool(name="xtp", bufs=1))
        qkv = ctx.enter_context(tc.tile_pool(name="qkv", bufs=1))
        big2 = ctx.enter_context(tc.tile_pool(name="big2", bufs=1))
        ep = ctx.enter_context(tc.tile_pool(name="ep", bufs=6))
        rp = ctx.enter_context(tc.tile_pool(name="rp", bufs=3))
        yp = ctx.enter_context(tc.tile_pool(name="yp", bufs=4))

        # ---- SBUF destination tiles ----
        wqk_sb = consts.tile([128, 4, DC, 128], bf16)
        wv_sb = consts.tile([128, DC, 256], bf16)
        wo_sb = consts.tile([128, 2, D], bf16)
        covT_sb = consts.tile([1, n], bf16)
        wce1_sb = consts.tile([1, 256], bf16)
        bce1_sb = consts.tile([128, 2], f32)
        wce2_sb = consts.tile([128, 8], bf16)
        bce2_sb = consts.tile([128, 4], f32)
        wfg1_sb = consts.tile([128, DC, 256], bf16)
        bfg1_sb = consts.tile([128, 2], f32)
        wfg2_sb = consts.tile([128, 2], f32)
        bfg2_sb = consts.tile([1, 1], f32)
        xts = xtp.tile([128, 2, DC, 1024], bf16, name="xts")

        # ---- DMA schedule: two HW rings, time-critical first ----
        # sync : wqk(K0) -> x(jc0,lo) -> x(jc1,lo) -> wqk(Q0) -> wo
        # scalar: smalls -> x(jc0,hi) -> wv -> wqk(K1,Q1) -> x(jc1,hi) -> wfg1
        nc.sync.dma_start(out=wqk_sb[:, 2], in_=WQK[:, 2])
        nc.sync.dma_start(out=xts[:, 0, 0:4], in_=XT[:, 0, 0:4])
        nc.sync.dma_start(out=xts[:, 1, 0:4], in_=XT[:, 1, 0:4])
        nc.sync.dma_start(out=wqk_sb[:, 0], in_=WQK[:, 0])
        nc.sync.dma_start(out=wo_sb, in_=WO)

        nc.scalar.dma_start(out=covT_sb, in_=COVT)
        nc.scalar.dma_start(out=wce1_sb, in_=WCE1)
        nc.scalar.dma_start(out=bce1_sb, in_=BCE1)
        nc.scalar.dma_start(out=wce2_sb, in_=WCE2)
        nc.scalar.dma_start(out=bce2_sb, in_=BCE2)
        nc.scalar.dma_start(out=bfg1_sb, in_=BFG1)
        nc.scalar.dma_start(out=wfg2_sb, in_=WFG2)
        nc.scalar.dma_start(out=bfg2_sb, in_=BFG2)
        nc.scalar.dma_start(out=xts[:, 0, 4:8], in_=XT[:, 0, 4:8])
        nc.scalar.dma_start(out=wv_sb, in_=WV)
        nc.scalar.dma_start(out=wqk_sb[:, 3], in_=WQK[:, 3])
        nc.scalar.dma_start(out=wqk_sb[:, 1], in_=WQK[:, 1])
        nc.scalar.dma_start(out=xts[:, 1, 4:8], in_=XT[:, 1, 4:8])
        nc.scalar.dma_start(out=wfg1_sb, in_=WFG1)

        ones_f = consts.tile([1, 128], f32)
        nc.vector.memset(ones_f, 1.0)

        pooled4 = consts.tile([128, DC, 2], f32)
        pooled_sb = consts.tile([128, DC], f32)
        pooled_bf = consts.tile([128, DC], bf16)
        hidg_sb = consts.tile([128, 2], f32)
        g_sb = consts.tile([1, 1], f32)
        gb_sb = consts.tile([128, 1], f32)
        bias_sb = consts.tile([128, NJ, 4], f32)
        bias_dve = consts.tile([128, NJ, 4], f32)

        # pooled partial sums (free-axis reduce is DVE-only)
        for jc2 in range(2):
            for dc in range(DC):
                nc.vector.reduce_sum(pooled4[:, dc, jc2:jc2 + 1], xts[:, jc2, dc, :],
                                     axis=mybir.AxisListType.X)
        for dc in range(DC):
            nc.vector.reduce_sum(pooled_sb[:, dc:dc + 1], pooled4[:, dc, :],
                                 axis=mybir.AxisListType.X)
        nc.vector.tensor_copy(pooled_bf, pooled_sb)

        qt_sb = qkv.tile([128, 2, n], bf16)
        ktp_sb = qkv.tile([128, 2, n], bf16)
        vaug_sb = qkv.tile([128, NJ, 4, 65], bf16)
        nc.vector.memset(vaug_sb, 1.0)

        # ---- prefix: cov MLP, K0, Q0, V, K1, Q1, gate MLP ----
        with tc.tile_pool(name="pfA", bufs=3, space="PSUM") as pfA, \
             tc.tile_pool(name="pft", bufs=1, space="PSUM") as pft:

            # coverage MLP (tiny): hidden^T then per-jt bias
            hidc_sb = big2.tile([128, 2, n], bf16, tag="big", name="hidc")
            for mc in range(2):
                for jc in range(NI):
                    ph = pft.tile([128, 512], f32, tag="tiny")
                    nc.tensor.matmul(ph, wce1_sb[:, mc * 128:(mc + 1) * 128],
                                     covT_sb[:, jc * 512:(jc + 1) * 512],
                                     start=True, stop=True)
                    nc.scalar.activation(out=hidc_sb[:, mc, jc * 512:(jc + 1) * 512],
                                         in_=ph, func=AFT.Silu,
                                         bias=bce1_sb[:, mc:mc + 1], scale=1.0)
            for jt in range(NJ):
                pc = pft.tile([128, 512], f32, tag="tiny")
                for mc in range(2):
                    nc.tensor.matmul(pc[:, 0:4], hidc_sb[:, mc, jt * 128:(jt + 1) * 128],
                                     wce2_sb[:, mc * 4:(mc + 1) * 4],
                                     start=(mc == 0), stop=(mc == 1))
                nc.vector.tensor_add(bias_sb[:, jt, :], pc[:, 0:4], bce2_sb)

            def qk_chunk(cb, ic, i):
                pq = pfA.tile([128, 512], f32, tag="qk", name=f"pq{cb}_{ic}",
                              bufs=2)
                jc2, sub = ic // 2, (ic % 2) * 512
                for dc in range(DC):
                    nc.tensor.matmul(pq, wqk_sb[:, cb, dc, :],
                                     xts[:, jc2, dc, sub:sub + 512],
                                     start=(dc == 0), stop=(dc == DC - 1))
                dst = (ktp_sb[:, cb - 2, ic * 512:(ic + 1) * 512] if cb >= 2
                       else qt_sb[:, cb, ic * 512:(ic + 1) * 512])
                eng = (nc.vector, nc.scalar, nc.vector, nc.scalar)[i % 4]
                if eng is nc.scalar:
                    eng.copy(dst, pq)
                else:
                    eng.tensor_copy(dst, pq)

            def v_chunk(it):
                pv = pfA.tile([128, 4, 64], f32, tag="v", name=f"pv{it}", bufs=2)
                jc2, col = it // 8, (it % 8) * 128
                for dc in range(DC):
                    nc.tensor.matmul(pv, xts[:, jc2, dc, col:col + 128],
                                     wv_sb[:, dc, :],
                                     start=(dc == 0), stop=(dc == DC - 1))
                if it % 2 == 0:
                    nc.vector.tensor_copy(vaug_sb[:, it, :, 0:64], pv)
                else:
                    nc.scalar.copy(vaug_sb[:, it, :, 0:64], pv)

            i = 0
            for ic in range(NI):
                qk_chunk(2, ic, i); i += 1
            for ic in range(NI):
                qk_chunk(0, ic, i); i += 1
            for it in range(NJ):
                v_chunk(it)
            for ic in range(NI):
                qk_chunk(3, ic, i); i += 1
            for ic in range(NI):
                qk_chunk(1, ic, i); i += 1

            # gate MLP (bf16 matmul on pooled sums)
            pg = pft.tile([128, 512], f32, tag="tiny", name="pg")
            for mc in range(2):
                for dc in range(DC):
                    nc.tensor.matmul(pg[:, mc:mc + 1],
                                     wfg1_sb[:, dc, mc * 128:(mc + 1) * 128],
                                     pooled_bf[:, dc:dc + 1],
                                     start=(dc == 0), stop=(dc == DC - 1))
            for mc in range(2):
                nc.scalar.activation(out=hidg_sb[:, mc:mc + 1], in_=pg[:, mc:mc + 1],
                                     func=AFT.Silu, bias=bfg1_sb[:, mc:mc + 1],
                                     scale=1.0 / n)
            pgp = pft.tile([128, 512], f32, tag="tiny")
            for mc in range(2):
                nc.tensor.matmul(pgp[0:1, 0:1], hidg_sb[:, mc:mc + 1],
                                 wfg2_sb[:, mc:mc + 1],
                                 start=(mc == 0), stop=(mc == 1))
            nc.scalar.activation(out=g_sb, in_=pgp[0:1, 0:1], func=AFT.Sigmoid,
                                 bias=bfg2_sb, scale=1.0)
            pgb = pft.tile([128, 512], f32, tag="tiny")
            nc.tensor.matmul(pgb[:, 0:1], ones_f, g_sb, start=True, stop=True)
            nc.vector.tensor_copy(gb_sb, pgb[:, 0:1])

            # bias_sb *= g ; bias_dve = bias_sb*A + B (for DVE schraudolph)
            nc.vector.tensor_scalar_mul(out=bias_sb[:, :, :],
                                        in0=bias_sb[:, :, :], scalar1=gb_sb)
            scale = float(HD) ** -0.5
            nc.vector.tensor_scalar(out=bias_dve[:, :, :], in0=bias_sb[:, :, :],
                                    scalar1=SCHRAUD_A, scalar2=SCHRAUD_B,
                                    op0=ALU.mult, op1=ALU.add)
            # exp table warmup so the first stream exp pays no table load
            warm_sb = consts.tile([1, 128], bf16)
            nc.scalar.activation(out=warm_sb, in_=pgb[0:1, 0:128], func=AFT.Exp,
                                 scale=0.001)

        # ---- attention stream: software-pipelined per (p, ic) ----
        attn_sb = big2.tile([128, 2, n], bf16, tag="big", name="attn")
        with tc.tile_pool(name="pop", bufs=2, space="PSUM") as pop, \
             tc.tile_pool(name="pss", bufs=2, space="PSUM") as pss:

            def s_tiles(p, ic, jt):
                """Emit S^T matmuls for (p, ic, jt); returns the 2 PSUM tiles."""
                out = []
                js = slice(jt * 128, (jt + 1) * 128)
                for hh in range(2):
                    lo = hh * 64
                    ps_ = pss.tile([128, IC_W], f32, tag="s",
                                   name=f"s{p}_{ic}_{jt}_{hh}")
                    for q in range(IC_W // 512):
                        nc.tensor.matmul(
                            ps_[:, q * 512:(q + 1) * 512],
                            ktp_sb[lo:lo + 64, p, js],
                            qt_sb[lo:lo + 64, p,
                                  ic * IC_W + q * 512:ic * IC_W + (q + 1) * 512],
                            start=True, stop=True)
                    out.append(ps_)
                return out

            scale = float(HD) ** -0.5
            for p in range(2):
                for ic in range(NI2):
                    po = [pop.tile([65, IC_W], f32, tag="o",
                                   name=f"po{p}_{ic}_{i}") for i in range(2)]
                    pipe = not bool(int(_os.environ.get("KPIPE_OFF", "0")))
                    if pipe:
                        pend = s_tiles(p, ic, 0)
                    for jt in range(NJ):
                        if not pipe:
                            pend = s_tiles(p, ic, jt)
                        # exp(jt): scalar or DVE schraudolph
                        es = []
                        for hh in range(2):
                            h = 2 * p + hh
                            e = ep.tile([128, IC_W], bf16, tag="e",
                                        name=f"e{p}_{ic}_{jt}_{hh}")
                            if _dve_tile(jt, hh):
                                nc.vector.tensor_scalar(
                                    out=e.bitcast(i16), in0=pend[hh],
                                    scalar1=SCHRAUD_A * scale,
                                    scalar2=bias_dve[:, jt, h:h + 1],
                                    op0=ALU.mult, op1=ALU.add)
                            else:
                                nc.scalar.activation(out=e, in_=pend[hh],
                                                     func=AFT.Exp,
                                                     bias=bias_sb[:, jt, h:h + 1],
                                                     scale=scale)
                            es.append(e)
                        # S(jt+1) ahead of AV(jt) keeps the PE queue fed
                        if pipe and jt + 1 < NJ:
                            pend = s_tiles(p, ic, jt + 1)
                        st, sp = (jt == 0), (jt == NJ - 1)
                        for hh in range(2):
                            h = 2 * p + hh
                            for q in range(IC_W // 512):
                                nc.tensor.matmul(
                                    po[hh][:, q * 512:(q + 1) * 512],
                                    vaug_sb[:, jt, h, :],
                                    es[hh][:, q * 512:(q + 1) * 512],
                                    start=st, stop=sp)
                    # normalize: O^T rows 0..63, denominator row 64
                    osl = slice(ic * IC_W, (ic + 1) * IC_W)
                    for hh in range(2):
                        lo = hh * 64
                        # NB: reciprocal_approx_fast misreads a partition-64
                        # PSUM AP; copy the den row to partition 0 first.
                        dd = rp.tile([1, IC_W], f32, tag="dd",
                                     name=f"dd{p}_{ic}_{hh}")
                        nc.vector.tensor_copy(dd, po[hh][64:65, :])
                        rr = rp.tile([1, IC_W], f32, tag="rr",
                                     name=f"rr{p}_{ic}_{hh}")
                        nc.vector.reciprocal_approx_fast(out=rr, in_=dd)
                        recb = rp.tile([64, IC_W], f32, tag="recb",
                                       name=f"recb{p}_{ic}_{hh}")
                        nc.gpsimd.partition_broadcast(recb, rr)
                        nc.vector.tensor_mul(attn_sb[lo:lo + 64, p, osl],
                                             po[hh][0:64, :], recb)

        if dbg:
            nc.sync.dma_start(out=DBG_Q, in_=qt_sb)
            nc.sync.dma_start(out=DBG_K, in_=ktp_sb)
            nc.sync.dma_start(out=DBG_V, in_=vaug_sb)
            nc.sync.dma_start(out=DBG_B, in_=bias_sb)
            nc.sync.dma_start(out=DBG_A, in_=attn_sb)

        # ---- tail: out-projection ----
        with tc.tile_pool(name="psy", bufs=2, space="PSUM") as psy:
            for it in range(NJ):
                py = psy.tile([128, D], f32, tag="y")
                for pt in range(2):
                    for half in range(2):
                        nc.tensor.matmul(
                            py[:, half * 512:(half + 1) * 512],
                            attn_sb[:, pt, it * 128:(it + 1) * 128],
                            wo_sb[:, pt, half * 512:(half + 1) * 512],
                            start=(pt == 0), stop=(pt == 1))
                y_sb = yp.tile([128, D], bf16, tag="y_sb", name=f"ysb{it}")
                if it % 2 == 0:
                    nc.vector.tensor_copy(y_sb, py)
                else:
                    nc.scalar.copy(y_sb, py)
                eng = nc.sync if it % 2 == 0 else nc.scalar
                eng.dma_start(out=OUT[it * 128:(it + 1) * 128, :], in_=y_sb)

    nc.compile()
    return nc


def make_in_maps(x, coverage, w_qkv, w_out, b_out, w_ce1, b_ce1, w_ce2, b_ce2,
                 w_fg1, b_fg1, w_fg2, b_fg2, n=N):
    f = np.float32
    DC = D // 128
    x = np.asarray(x, f)
    coverage = np.asarray(coverage, f)
    w_qkv = np.asarray(w_qkv, f)
    w_out = np.asarray(w_out, f)

    def pmajor(a, inner):
        # [(blocks*128), inner] -> [128, blocks, inner]
        blocks = a.shape[0] // 128
        return np.ascontiguousarray(
            a.reshape(blocks, 128, inner).transpose(1, 0, 2))

    in_maps = []
    for c in range(NCORES):
        b, hg = divmod(c, 4)
        cs, ce = hg * 256, (hg + 1) * 256
        wq = w_qkv[:, 0 * D + cs:0 * D + ce]
        wk = w_qkv[:, 1 * D + cs:1 * D + ce]
        wv = w_qkv[:, 2 * D + cs:2 * D + ce]
        # wqk packed (p, cb, dc, 128): cb0/1 = Q cols 0:128/128:256, cb2/3 = K
        wqk4 = np.concatenate([wq, wk], axis=1)          # (1024, 512)
        wqk4 = wqk4.reshape(DC, 128, 4, 128).transpose(1, 2, 0, 3)  # p cb dc col
        # xT packed (p, jc2, dc, tok): element = x[b][tok, dc*128+p]
        xt = x[b].T.reshape(DC, 128, 2, 1024).transpose(1, 2, 0, 3)
        m = {
            "xT": _bf16(xt),
            "wqk": _bf16(wqk4),
            "wv": _bf16(pmajor(wv, 256)),
            "wo": _bf16(pmajor(w_out[cs:ce, :], D)),
            "covT": _bf16(coverage[b, :, 0][None, :]),
            "wce1": _bf16(w_ce1),
            "bce1": np.ascontiguousarray(np.asarray(b_ce1, f).reshape(2, 128).T),
            "wce2": _bf16(
                np.asarray(w_ce2, f)[:, 4 * hg:4 * hg + 4].reshape(2, 128, 4)
                .transpose(1, 0, 2).reshape(128, 8)),
            "bce2": np.tile(np.asarray(b_ce2, f)[4 * hg:4 * hg + 4][None, :], (128, 1)),
            "wfg1": _bf16(pmajor(np.asarray(w_fg1, f), 256)),
            "bfg1": np.ascontiguousarray(np.asarray(b_fg1, f).reshape(2, 128).T),
            "wfg2": np.ascontiguousarray(np.asarray(w_fg2, f).reshape(2, 128).T),
            "bfg2": np.asarray(b_fg2, f).reshape(1, 1),
        }
        in_maps.append(m)
    return in_maps


def kernel(**inputs):
    from concourse.bass_utils import run_bass_kernel_spmd
    if "nc" not in _COMPILED:
        _COMPILED["nc"] = build(N)
    nc = _COMPILED["nc"]
    in_maps = make_in_maps(**inputs)
    res = run_bass_kernel_spmd(nc, in_maps, core_ids=list(range(NCORES)))
    outs = [np.asarray(res.results[c]["out"], dtype=np.float32)
            for c in range(NCORES)]
    b_out = np.asarray(inputs["b_out"], np.float32)
    full = np.stack([
        outs[0] + outs[1] + outs[2] + outs[3] + b_out[None, :],
        outs[4] + outs[5] + outs[6] + outs[7] + b_out[None, :],
    ]).astype(np.float32)
    return full


# revision 16
# speedup vs baseline: 1.1663x; 1.0472x over previous
"""AdaptiveCoverageAttention TRN2 kernel: 8-way (batch x head-group) sharded.

Sharding: core c in 0..7 -> batch b = c//4, head-group hg = c%4 (4 heads each).
Each core computes its 4 heads' attention + its partial output projection;
the host sums the 4 partials per batch (and adds b_out). No collectives.

v5: PE-roofline oriented (PE ~393k cycles/core @2.4GHz = 164us).
- IC_W=512: every stream PSUM tile is one bank. pss bufs=3 gives the
  S->exp->S chain 1.5 iterations of slack; po bufs=4 gives normalize a
  whole block of slack. Job pool (1 bank) hosts all projection/MLP/out-proj
  matmuls INTERLEAVED into the stream so the in-order PE queue never
  drains (keeps the PE DVFS p-state at 2.4GHz).
- exp tiles [128,512]: hh1/jt-odd quarter runs on VectorE via Schraudolph
  bf16 (int16 convert + bitcast, mean-centered C=-7.37, ~+7e-3 rel err),
  rest on ScalarE.
- Pooled sums for the gate MLP: half on DVE reduce, half via ScalarE
  activation accum_out, so the gate (which gates the first exp) is ready
  ~23us in.
- Host pre-packs everything partition-major; ~17 large DMAs on the two
  HW DGE rings, small consts packed into 3 DMAs.
- Normalize per (p,ic): dd copy + reciprocal on DVE (recip misreads
  partition-offset PSUM APs, so copy to partition 0 first), broadcast on
  GpSimd, mul on DVE.
- Out-projection runs as jobs after both pairs of an i-range normalize;
  last block's 4 tiles in a short tail.
"""
import os as _os
import sys

sys.path.insert(0, "/opt/trn_rl_repo")

import numpy as np

B, N, D, H = 2, 2048, 1024, 16
HD = D // H            # 64
NCORES = 8
IC_W = 512

_COMPILED = {}

SCHRAUD_A = float(128.0 * np.log2(np.e))
SCHRAUD_B = float(127.0 * 128.0 - 7.37)
_DVE_OFF = bool(int(_os.environ.get("KDVE_OFF", "0")))


def _bf16(x):
    import ml_dtypes
    return np.ascontiguousarray(np.asarray(x, np.float32)).astype(ml_dtypes.bfloat16)


def _dve_tile(jt, hh):
    """Which exp tiles run on VectorE (Schraudolph). 25% of tiles."""
    if _DVE_OFF:
        return False
    return hh == 1 and jt % 2 == 1


def build(n=N):
    import contextlib

    import concourse.bacc as bacc
    import concourse.tile as tile
    from concourse import mybir

    f32 = mybir.dt.float32
    bf16 = mybir.dt.bfloat16
    i16 = mybir.dt.int16
    AFT = mybir.ActivationFunctionType
    ALU = mybir.AluOpType

    NJ = n // 128          # 16 j-tiles
    NI = n // 512          # 4 i-chunks of 512 (also = stream blocks/pair)
    DC = D // 128          # 8 contraction chunks
    scale = float(HD) ** -0.5

    nc = bacc.Bacc("TRN2", target_bir_lowering=False, debug=False,
                   num_devices=NCORES)

    dram = lambda name, shape, dt, kind: nc.dram_tensor(name, shape, dt, kind=kind).ap()
    XT = dram("xT", [128, 2, DC, 1024], bf16, "ExternalInput")     # (p, jc2, dc, tok)
    WQK = dram("wqk", [128, 4, DC, 128], bf16, "ExternalInput")    # (p, cb, dc, col)
    WV = dram("wv", [128, DC, 256], bf16, "ExternalInput")
    WO = dram("wo", [128, 2, D], bf16, "ExternalInput")
    CVW = dram("cvw", [1, n + 256], bf16, "ExternalInput")         # covT | wce1
    WCE2 = dram("wce2", [128, 8], bf16, "ExternalInput")
    SMF = dram("smf", [128, 11], f32, "ExternalInput")  # bce1|bce2|bfg1|wfg2|bfg2
    WFG1 = dram("wfg1", [128, DC, 256], bf16, "ExternalInput")
    OUT = dram("out", [n, D], bf16, "ExternalOutput")
    dbg = bool(int(_os.environ.get("KDBG", "0")))
    if dbg:
        DBG_Q = dram("dbg_q", [128, 2, n], bf16, "ExternalOutput")
        DBG_K = dram("dbg_k", [128, 2, n], bf16, "ExternalOutput")
        DBG_V = dram("dbg_v", [128, NJ, 4, 65], bf16, "ExternalOutput")
        DBG_B = dram("dbg_b", [128, NJ, 4], f32, "ExternalOutput")
        DBG_A = dram("dbg_a", [128, 2, n], bf16, "ExternalOutput")

    with tile.TileContext(nc) as tc, contextlib.ExitStack() as ctx:
        consts = ctx.enter_context(tc.tile_pool(name="consts", bufs=1))
        xtp = ctx.enter_context(tc.tile_pool(name="xtp", bufs=1))
        qkv = ctx.enter_context(tc.tile_pool(name="qkv", bufs=1))
        big2 = ctx.enter_context(tc.tile_pool(name="big2", bufs=1))
        ep = ctx.enter_context(tc.tile_pool(name="ep", bufs=8))
        rp = ctx.enter_context(tc.tile_pool(name="rp", bufs=4))
        yp = ctx.enter_context(tc.tile_pool(name="yp", bufs=6))

        wqk_sb = consts.tile([128, 4, DC, 128], bf16)
        wv_sb = consts.tile([128, DC, 256], bf16)
        wo_sb = consts.tile([128, 2, D], bf16)
        cvw_sb = consts.tile([1, n + 256], bf16)
        covT_sb = cvw_sb[:, 0:n]
        wce1_sb = cvw_sb[:, n:n + 256]
        wce2_sb = consts.tile([128, 8], bf16)
        smf_sb = consts.tile([128, 11], f32)
        bce1_sb = smf_sb[:, 0:2]
        bce2_sb = smf_sb[:, 2:6]
        bfg1_sb = smf_sb[:, 6:8]
        wfg2_sb = smf_sb[:, 8:10]
        bfg2_sb = smf_sb[0:1, 10:11]
        wfg1_sb = consts.tile([128, DC, 256], bf16)
        xts = xtp.tile([128, 2, DC, 1024], bf16, name="xts")

        # ---- DMA schedule: two HW rings, time-critical first ----
        nc.sync.dma_start(out=wqk_sb[:, 2], in_=WQK[:, 2])       # K0
        nc.sync.dma_start(out=xts[:, 0, 0:4], in_=XT[:, 0, 0:4])
        nc.sync.dma_start(out=wqk_sb[:, 0], in_=WQK[:, 0])       # Q0
        nc.sync.dma_start(out=xts[:, 1, 0:4], in_=XT[:, 1, 0:4])
        nc.sync.dma_start(out=wo_sb, in_=WO)

        nc.scalar.dma_start(out=xts[:, 0, 4:8], in_=XT[:, 0, 4:8])
        nc.scalar.dma_start(out=cvw_sb, in_=CVW)
        nc.scalar.dma_start(out=smf_sb, in_=SMF)
        nc.scalar.dma_start(out=wce2_sb, in_=WCE2)
        nc.scalar.dma_start(out=wv_sb, in_=WV)
        nc.scalar.dma_start(out=wqk_sb[:, 3], in_=WQK[:, 3])     # K1
        nc.scalar.dma_start(out=wqk_sb[:, 1], in_=WQK[:, 1])     # Q1
        nc.scalar.dma_start(out=xts[:, 1, 4:8], in_=XT[:, 1, 4:8])
        nc.scalar.dma_start(out=wfg1_sb, in_=WFG1)

        ones_f = consts.tile([1, 128], f32)
        nc.vector.memset(ones_f, 1.0)

        pooled4 = consts.tile([128, DC, 2], f32)
        pooled_sb = consts.tile([128, DC], f32)
        pooled_bf = consts.tile([128, DC], bf16)
        trash = consts.tile([128, 1024], f32)
        hidg_sb = consts.tile([128, 2], f32)
        g_sb = consts.tile([1, 1], f32)
        gb_sb = consts.tile([128, 1], f32)
        bias_sb = consts.tile([128, NJ, 4], f32)
        bias_dve = consts.tile([128, NJ, 4], f32)

        # pooled partial sums: half DVE reduce, half ScalarE accum_out
        for jc2 in range(2):
            for dc in range(DC):
                if dc % 2 == 0:
                    nc.vector.reduce_sum(pooled4[:, dc, jc2:jc2 + 1],
                                         xts[:, jc2, dc, :],
                                         axis=mybir.AxisListType.X)
                else:
                    nc.scalar.activation(out=trash, in_=xts[:, jc2, dc, :],
                                         func=AFT.Copy, scale=1.0,
                                         accum_out=pooled4[:, dc, jc2:jc2 + 1])
        for dc in range(DC):
            nc.vector.reduce_sum(pooled_sb[:, dc:dc + 1], pooled4[:, dc, :],
                                 axis=mybir.AxisListType.X)
        nc.vector.tensor_copy(pooled_bf, pooled_sb)

        qt_sb = qkv.tile([128, 2, n], bf16)
        ktp_sb = qkv.tile([128, 2, n], bf16)
        vaug_sb = qkv.tile([128, NJ, 4, 65], bf16)
        nc.vector.memset(vaug_sb, 1.0)
        hidc_sb = big2.tile([128, 2, n], bf16, tag="big", name="hidc")
        attn_sb = big2.tile([128, 2, n], bf16, tag="big", name="attn")

        # ================= stream with interleaved jobs =================
        with tc.tile_pool(name="pss", bufs=3, space="PSUM") as pss, \
             tc.tile_pool(name="pop", bufs=4, space="PSUM") as pop, \
             tc.tile_pool(name="pj", bufs=1, space="PSUM") as pj:

            cp_i = [0]

            def cp_eng():
                cp_i[0] += 1
                return nc.vector if cp_i[0] % 2 == 0 else nc.scalar

            def copy(eng, dst, src):
                if eng is nc.scalar:
                    eng.copy(dst, src)
                else:
                    eng.tensor_copy(dst, src)

            def qk_job(cb, ic):
                pq = pj.tile([128, 512], f32, tag="job", name=f"pq{cb}_{ic}")
                jc2, sub = ic // 2, (ic % 2) * 512
                for dc in range(DC):
                    nc.tensor.matmul(pq, wqk_sb[:, cb, dc, :],
                                     xts[:, jc2, dc, sub:sub + 512],
                                     start=(dc == 0), stop=(dc == DC - 1))
                dst = (ktp_sb[:, cb - 2, ic * 512:(ic + 1) * 512] if cb >= 2
                       else qt_sb[:, cb, ic * 512:(ic + 1) * 512])
                copy(cp_eng(), dst, pq)

            def v_job(it):
                pv = pj.tile([128, 4, 64], f32, tag="job", name=f"pv{it}")
                jc2, col = it // 8, (it % 8) * 128
                for dc in range(DC):
                    nc.tensor.matmul(pv, xts[:, jc2, dc, col:col + 128],
                                     wv_sb[:, dc, :],
                                     start=(dc == 0), stop=(dc == DC - 1))
                copy(cp_eng(), vaug_sb[:, it, :, 0:64], pv)

            def covh_job(mc, q):
                ph = pj.tile([128, 512], f32, tag="job", name=f"ph{mc}_{q}")
                nc.tensor.matmul(ph, wce1_sb[:, mc * 128:(mc + 1) * 128],
                                 covT_sb[:, q * 512:(q + 1) * 512],
                                 start=True, stop=True)
                nc.scalar.activation(
                    out=hidc_sb[:, mc, q * 512:(q + 1) * 512],
                    in_=ph, func=AFT.Silu, bias=bce1_sb[:, mc:mc + 1], scale=1.0)

            def covb_job(jt4):
                pc = pj.tile([128, 512], f32, tag="job", name=f"pc{jt4}")
                for k in range(4):
                    jt = jt4 * 4 + k
                    for mc in range(2):
                        nc.tensor.matmul(pc[:, k * 4:k * 4 + 4],
                                         hidc_sb[:, mc, jt * 128:(jt + 1) * 128],
                                         wce2_sb[:, mc * 4:(mc + 1) * 4],
                                         start=(mc == 0), stop=(mc == 1))
                for k in range(4):
                    jt = jt4 * 4 + k
                    nc.vector.tensor_add(bias_sb[:, jt, :], pc[:, k * 4:k * 4 + 4],
                                         bce2_sb)

            def gate_job():
                pg = pj.tile([128, 512], f32, tag="job", name="pg")
                for mc in range(2):
                    for dc in range(DC):
                        nc.tensor.matmul(pg[:, mc:mc + 1],
                                         wfg1_sb[:, dc, mc * 128:(mc + 1) * 128],
                                         pooled_bf[:, dc:dc + 1],
                                         start=(dc == 0), stop=(dc == DC - 1))
                for mc in range(2):
                    nc.scalar.activation(out=hidg_sb[:, mc:mc + 1],
                                         in_=pg[:, mc:mc + 1], func=AFT.Silu,
                                         bias=bfg1_sb[:, mc:mc + 1], scale=1.0 / n)
                pgp = pj.tile([128, 512], f32, tag="job", name="pgp")
                for mc in range(2):
                    nc.tensor.matmul(pgp[0:1, 0:1], hidg_sb[:, mc:mc + 1],
                                     wfg2_sb[:, mc:mc + 1],
                                     start=(mc == 0), stop=(mc == 1))
                nc.scalar.activation(out=g_sb, in_=pgp[0:1, 0:1], func=AFT.Sigmoid,
                                     bias=bfg2_sb, scale=1.0)
                pgb = pj.tile([128, 512], f32, tag="job", name="pgb")
                nc.tensor.matmul(pgb[:, 0:1], ones_f, g_sb, start=True, stop=True)
                nc.vector.tensor_copy(gb_sb, pgb[:, 0:1])
                nc.vector.tensor_scalar_mul(out=bias_sb[:, :, :],
                                            in0=bias_sb[:, :, :], scalar1=gb_sb)
                nc.vector.tensor_scalar(out=bias_dve[:, :, :], in0=bias_sb[:, :, :],
                                        scalar1=SCHRAUD_A, scalar2=SCHRAUD_B,
                                        op0=ALU.mult, op1=ALU.add)
                # exp table warmup
                warm = consts.tile([1, 128], bf16)
                nc.scalar.activation(out=warm, in_=pgb[0:1, 0:128], func=AFT.Exp,
                                     scale=0.001)

            def oproj_job(it, half):
                py = pj.tile([128, 512], f32, tag="job", name=f"py{it}_{half}")
                for pt in range(2):
                    nc.tensor.matmul(py, attn_sb[:, pt, it * 128:(it + 1) * 128],
                                     wo_sb[:, pt, half * 512:(half + 1) * 512],
                                     start=(pt == 0), stop=(pt == 1))
                y_sb = yp.tile([128, 512], bf16, tag="y", name=f"y{it}_{half}")
                copy(cp_eng(), y_sb, py)
                nc.sync.dma_start(out=OUT[it * 128:(it + 1) * 128,
                                          half * 512:(half + 1) * 512], in_=y_sb)

            # popped 1/iter DURING the stream (emitted between S and AV).
            # Ordering rule: every producer must be EMITTED before its
            # first consumer (program-order read-before-write is a race):
            # Q0ic1 before block 1, Q0ic2 before block 2, K1/Q1 before
            # block 4, v_job(it) before AV(jt=it) of block 0.
            jobs = []
            jobs += [lambda it=it: v_job(it) for it in range(4, 10)]
            jobs.append(lambda: qk_job(0, 1))
            jobs += [lambda it=it: v_job(it) for it in range(10, 16)]
            jobs += [lambda ic=ic: qk_job(3, ic) for ic in range(3)]
            jobs.append(lambda: qk_job(0, 2))
            jobs.append(lambda: qk_job(3, 3))
            jobs.append(lambda: qk_job(1, 0))
            jobs.append(lambda: qk_job(0, 3))
            jobs += [lambda ic=ic: qk_job(1, ic) for ic in range(1, 4)]
            jobs.reverse()   # pop() from the end

            def s_tiles(p, ic, jt):
                out = []
                js = slice(jt * 128, (jt + 1) * 128)
                for hh in range(2):
                    lo = hh * 64
                    ps_ = pss.tile([128, IC_W], f32, tag="s",
                                   name=f"s{p}_{ic}_{jt}_{hh}")
                    nc.tensor.matmul(ps_, ktp_sb[lo:lo + 64, p, js],
                                     qt_sb[lo:lo + 64, p,
                                           ic * IC_W:(ic + 1) * IC_W],
                                     start=True, stop=True)
                    out.append(ps_)
                return out

            # pre-stream: K0 ic0 + Q0 ic0 so block 1 can start, then the
            # work that feeds bias_sb (everything the first exp needs MUST
            # precede the first AV in the in-order PE queue, or it
            # deadlocks behind it), padded with early jobs.
            qk_job(2, 0)
            qk_job(0, 0)
            for q in range(4):
                covh_job(0, q)
                covh_job(1, q)
            qk_job(2, 1)
            qk_job(2, 2)
            qk_job(2, 3)
            for j in range(4):
                covb_job(j)
            for it in range(4):
                v_job(it)
            gate_job()

            blocks = [(p, ic) for p in range(2) for ic in range(NI)]
            for bi, (p, ic) in enumerate(blocks):
                po = [pop.tile([65, IC_W], f32, tag="o",
                               name=f"po{p}_{ic}_{i}") for i in range(2)]
                pend = s_tiles(p, ic, 0)
                for jt in range(NJ):
                    es = []
                    for hh in range(2):
                        h = 2 * p + hh
                        e = ep.tile([128, IC_W], bf16, tag="e",
                                    name=f"e{p}_{ic}_{jt}_{hh}")
                        if _dve_tile(jt, hh):
                            nc.vector.tensor_scalar(
                                out=e.bitcast(i16), in0=pend[hh],
                                scalar1=SCHRAUD_A * scale,
                                scalar2=bias_dve[:, jt, h:h + 1],
                                op0=ALU.mult, op1=ALU.add)
                        else:
                            nc.scalar.activation(out=e, in_=pend[hh],
                                                 func=AFT.Exp,
                                                 bias=bias_sb[:, jt, h:h + 1],
                                                 scale=scale)
                        es.append(e)
                    if jt + 1 < NJ:
                        pend = s_tiles(p, ic, jt + 1)
                    # job goes BEFORE the exp-gated AV so the PE queue
                    # always has runnable work at its head
                    if jobs:
                        jobs.pop()()
                    st, sp = (jt == 0), (jt == NJ - 1)
                    for hh in range(2):
                        h = 2 * p + hh
                        nc.tensor.matmul(po[hh], vaug_sb[:, jt, h, :], es[hh],
                                         start=st, stop=sp)
                # normalize: O^T rows 0..63, denominator row 64
                osl = slice(ic * IC_W, (ic + 1) * IC_W)
                for hh in range(2):
                    lo = hh * 64
                    dd = rp.tile([1, IC_W], f32, tag="dd", name=f"dd{p}_{ic}_{hh}")
                    nc.vector.tensor_copy(dd, po[hh][64:65, :])
                    rr = rp.tile([1, IC_W], f32, tag="rr", name=f"rr{p}_{ic}_{hh}")
                    nc.vector.reciprocal_approx_fast(out=rr, in_=dd)
                    recb = rp.tile([64, IC_W], f32, tag="recb",
                                   name=f"recb{p}_{ic}_{hh}")
                    nc.gpsimd.partition_broadcast(recb, rr)
                    nc.vector.tensor_mul(attn_sb[lo:lo + 64, p, osl],
                                         po[hh][0:64, :], recb)
                if p == 1 and ic < NI - 1:
                    for it in range(ic * 4, ic * 4 + 4):
                        jobs.append(lambda it=it, h=1: oproj_job(it, h))
                        jobs.append(lambda it=it, h=0: oproj_job(it, h))
            while jobs:
                jobs.pop()()

        if dbg:
            nc.sync.dma_start(out=DBG_Q, in_=qt_sb)
            nc.sync.dma_start(out=DBG_K, in_=ktp_sb)
            nc.sync.dma_start(out=DBG_V, in_=vaug_sb)
            nc.sync.dma_start(out=DBG_B, in_=bias_sb)
            nc.sync.dma_start(out=DBG_A, in_=attn_sb)

        # ---- tail: last block's out-projection with a wide pool ----
        with tc.tile_pool(name="psy", bufs=2, space="PSUM") as psy:
            for it in range(NJ - 4, NJ):
                py = psy.tile([128, D], f32, tag="y")
                for pt in range(2):
                    for half in range(2):
                        nc.tensor.matmul(
                            py[:, half * 512:(half + 1) * 512],
                            attn_sb[:, pt, it * 128:(it + 1) * 128],
                            wo_sb[:, pt, half * 512:(half + 1) * 512],
                            start=(pt == 0), stop=(pt == 1))
                y_sb = yp.tile([128, D], bf16, tag="yt", name=f"ysb{it}")
                if it % 2 == 0:
                    nc.vector.tensor_copy(y_sb, py)
                else:
                    nc.scalar.copy(y_sb, py)
                eng = nc.sync if it % 2 == 0 else nc.scalar
                eng.dma_start(out=OUT[it * 128:(it + 1) * 128, :], in_=y_sb)

    nc.compile()
    return nc


def make_in_maps(x, coverage, w_qkv, w_out, b_out, w_ce1, b_ce1, w_ce2, b_ce2,
                 w_fg1, b_fg1, w_fg2, b_fg2, n=N):
    f = np.float32
    DC = D // 128
    x = np.asarray(x, f)
    coverage = np.asarray(coverage, f)
    w_qkv = np.asarray(w_qkv, f)
    w_out = np.asarray(w_out, f)

    def pmajor(a, inner):
        blocks = a.shape[0] // 128
        return np.ascontiguousarray(
            a.reshape(blocks, 128, inner).transpose(1, 0, 2))

    smf = np.concatenate([
        np.asarray(b_ce1, f).reshape(2, 128).T,
        np.tile(np.asarray(b_ce2, f).reshape(1, 16)[:, 0:4], (128, 1)) * 0,  # per-core below
        np.asarray(b_fg1, f).reshape(2, 128).T,
        np.asarray(w_fg2, f).reshape(2, 128).T,
        np.full((128, 1), np.asarray(b_fg2, f).reshape(()), f),
    ], axis=1)

    in_maps = []
    for c in range(NCORES):
        b, hg = divmod(c, 4)
        cs, ce = hg * 256, (hg + 1) * 256
        wq = w_qkv[:, 0 * D + cs:0 * D + ce]
        wk = w_qkv[:, 1 * D + cs:1 * D + ce]
        wv = w_qkv[:, 2 * D + cs:2 * D + ce]
        wqk4 = np.concatenate([wq, wk], axis=1)
        wqk4 = wqk4.reshape(DC, 128, 4, 128).transpose(1, 2, 0, 3)
        xt = x[b].T.reshape(DC, 128, 2, 1024).transpose(1, 2, 0, 3)
        smf_c = smf.copy()
        smf_c[:, 2:6] = np.tile(
            np.asarray(b_ce2, f)[4 * hg:4 * hg + 4][None, :], (128, 1))
        m = {
            "xT": _bf16(xt),
            "wqk": _bf16(wqk4),
            "wv": _bf16(pmajor(wv, 256)),
            "wo": _bf16(pmajor(w_out[cs:ce, :], D)),
            "cvw": _bf16(np.concatenate([coverage[b, :, 0],
                                         np.asarray(w_ce1, f).reshape(-1)])[None, :]),
            "wce2": _bf16(
                np.asarray(w_ce2, f)[:, 4 * hg:4 * hg + 4].reshape(2, 128, 4)
                .transpose(1, 0, 2).reshape(128, 8)),
            "smf": smf_c,
            "wfg1": _bf16(pmajor(np.asarray(w_fg1, f), 256)),
        }
        in_maps.append(m)
    return in_maps


def kernel(**inputs):
    from concourse.bass_utils import run_bass_kernel_spmd
    if "nc" not in _COMPILED:
        _COMPILED["nc"] = build(N)
    nc = _COMPILED["nc"]
    in_maps = make_in_maps(**inputs)
    res = run_bass_kernel_spmd(nc, in_maps, core_ids=list(range(NCORES)))
    outs = [np.asarray(res.results[c]["out"], dtype=np.float32)
            for c in range(NCORES)]
    b_out = np.asarray(inputs["b_out"], np.float32)
    full = np.stack([
        outs[0] + outs[1] + outs[2] + outs[3] + b_out[None, :],
        outs[4] + outs[5] + outs[6] + outs[7] + b_out[None, :],
    ]).astype(np.float32)
    return full


# revision 26
# speedup vs baseline: 1.2317x; 1.0561x over previous
"""AdaptiveCoverageAttention TRN2 kernel: 8-way (batch x head-group) sharded.

Sharding: core c in 0..7 -> batch b = c//4, head-group hg = c%4 (4 heads each).
Each core computes its 4 heads' attention + its partial output projection;
the host sums the 4 partials per batch (and adds b_out). No collectives.

v5: PE-roofline oriented (PE ~393k cycles/core @2.4GHz = 164us).
- IC_W=512: every stream PSUM tile is one bank. pss bufs=3 gives the
  S->exp->S chain 1.5 iterations of slack; po bufs=4 gives normalize a
  whole block of slack. Job pool (1 bank) hosts all projection/MLP/out-proj
  matmuls INTERLEAVED into the stream so the in-order PE queue never
  drains (keeps the PE DVFS p-state at 2.4GHz).
- exp tiles [128,512]: hh1/jt-odd quarter runs on VectorE via Schraudolph
  bf16 (int16 convert + bitcast, mean-centered C=-7.37, ~+7e-3 rel err),
  rest on ScalarE.
- Pooled sums for the gate MLP: half on DVE reduce, half via ScalarE
  activation accum_out, so the gate (which gates the first exp) is ready
  ~23us in.
- Host pre-packs everything partition-major; ~17 large DMAs on the two
  HW DGE rings, small consts packed into 3 DMAs.
- Normalize per (p,ic): dd copy + reciprocal on DVE (recip misreads
  partition-offset PSUM APs, so copy to partition 0 first), broadcast on
  GpSimd, mul on DVE.
- Out-projection runs as jobs after both pairs of an i-range normalize;
  last block's 4 tiles in a short tail.
"""
import os as _os
import sys

sys.path.insert(0, "/opt/trn_rl_repo")

import numpy as np

B, N, D, H = 2, 2048, 1024, 16
HD = D // H            # 64
NCORES = 8
IC_W = 512

_COMPILED = {}

SCHRAUD_A = float(128.0 * np.log2(np.e))
SCHRAUD_B = float(127.0 * 128.0 - 7.37)
_DVE_OFF = bool(int(_os.environ.get("KDVE_OFF", "0")))


def _bf16(x):
    import ml_dtypes
    return np.ascontiguousarray(np.asarray(x, np.float32)).astype(ml_dtypes.bfloat16)


def _dve_tile(jt, hh):
    """Which exp tiles run on VectorE (Schraudolph). 50% of tiles: the
    hh1 tiles, so the hh0 PSUM-bank chain runs through ScalarE and the
    hh1 chain through VectorE, fully decoupled."""
    if _DVE_OFF:
        return False
    return hh == 1


def build(n=N):
    import contextlib

    import concourse.bacc as bacc
    import concourse.tile as tile
    from concourse import mybir

    f32 = mybir.dt.float32
    bf16 = mybir.dt.bfloat16
    i16 = mybir.dt.int16
    AFT = mybir.ActivationFunctionType
    ALU = mybir.AluOpType

    NJ = n // 128          # 16 j-tiles
    NI = n // 512          # 4 i-chunks of 512 (also = stream blocks/pair)
    DC = D // 128          # 8 contraction chunks
    scale = float(HD) ** -0.5

    nc = bacc.Bacc("TRN2", target_bir_lowering=False, debug=False,
                   num_devices=NCORES)

    dram = lambda name, shape, dt, kind: nc.dram_tensor(name, shape, dt, kind=kind).ap()
    XT = dram("xT", [128, 2, DC, 1024], bf16, "ExternalInput")     # (p, jc2, dc, tok)
    WQK = dram("wqk", [128, 4, DC, 128], bf16, "ExternalInput")    # (p, cb, dc, col)
    WV = dram("wv", [128, DC, 256], bf16, "ExternalInput")
    WO = dram("wo", [128, 2, D], bf16, "ExternalInput")
    CVW = dram("cvw", [1, n + 256], bf16, "ExternalInput")         # covT | wce1
    WCE2 = dram("wce2", [128, 8], bf16, "ExternalInput")
    SMF = dram("smf", [128, 11], f32, "ExternalInput")  # bce1|bce2|bfg1|wfg2|bfg2
    WFG1 = dram("wfg1", [128, DC, 256], bf16, "ExternalInput")
    OUT = dram("out", [n, D], f32, "ExternalOutput")
    dbg = bool(int(_os.environ.get("KDBG", "0")))
    if dbg:
        DBG_Q = dram("dbg_q", [128, 2, n], bf16, "ExternalOutput")
        DBG_K = dram("dbg_k", [128, 2, n], bf16, "ExternalOutput")
        DBG_V = dram("dbg_v", [128, NJ, 4, 65], bf16, "ExternalOutput")
        DBG_B = dram("dbg_b", [128, NJ, 4], f32, "ExternalOutput")
        DBG_A = dram("dbg_a", [128, 2, n], bf16, "ExternalOutput")

    with tile.TileContext(nc) as tc, contextlib.ExitStack() as ctx:
        consts = ctx.enter_context(tc.tile_pool(name="consts", bufs=1))
        xtp = ctx.enter_context(tc.tile_pool(name="xtp", bufs=1))
        qkv = ctx.enter_context(tc.tile_pool(name="qkv", bufs=1))
        big2 = ctx.enter_context(tc.tile_pool(name="big2", bufs=1))
        ep = ctx.enter_context(tc.tile_pool(name="ep", bufs=8))
        rp = ctx.enter_context(tc.tile_pool(name="rp", bufs=4))
        yp = ctx.enter_context(tc.tile_pool(name="yp", bufs=6))

        wqk_sb = consts.tile([128, 4, DC, 128], bf16)
        wv_sb = consts.tile([128, DC, 256], bf16)
        wo_sb = consts.tile([128, 2, D], bf16)
        cvw_sb = consts.tile([1, n + 256], bf16)
        covT_sb = cvw_sb[:, 0:n]
        wce1_sb = cvw_sb[:, n:n + 256]
        wce2_sb = consts.tile([128, 8], bf16)
        smf_sb = consts.tile([128, 11], f32)
        bce1_sb = smf_sb[:, 0:2]
        bce2_sb = smf_sb[:, 2:6]
        bfg1_sb = smf_sb[:, 6:8]
        wfg2_sb = smf_sb[:, 8:10]
        bfg2_sb = smf_sb[0:1, 10:11]
        wfg1_sb = consts.tile([128, DC, 256], bf16)
        xts = xtp.tile([128, 2, DC, 1024], bf16, name="xts")

        # ---- DMA schedule: two HW rings, time-critical first ----
        nc.sync.dma_start(out=wqk_sb[:, 2], in_=WQK[:, 2])       # K0
        nc.sync.dma_start(out=xts[:, 0, 0:4], in_=XT[:, 0, 0:4])
        nc.sync.dma_start(out=wqk_sb[:, 0], in_=WQK[:, 0])       # Q0
        nc.sync.dma_start(out=xts[:, 1, 0:4], in_=XT[:, 1, 0:4])
        nc.sync.dma_start(out=wo_sb, in_=WO)

        nc.scalar.dma_start(out=xts[:, 0, 4:8], in_=XT[:, 0, 4:8])
        nc.scalar.dma_start(out=cvw_sb, in_=CVW)
        nc.scalar.dma_start(out=smf_sb, in_=SMF)
        nc.scalar.dma_start(out=wce2_sb, in_=WCE2)
        nc.scalar.dma_start(out=wv_sb, in_=WV)
        nc.scalar.dma_start(out=wqk_sb[:, 3], in_=WQK[:, 3])     # K1
        nc.scalar.dma_start(out=wqk_sb[:, 1], in_=WQK[:, 1])     # Q1
        nc.scalar.dma_start(out=xts[:, 1, 4:8], in_=XT[:, 1, 4:8])
        nc.scalar.dma_start(out=wfg1_sb, in_=WFG1)

        ones_f = consts.tile([1, 128], f32)
        nc.vector.memset(ones_f, 1.0)

        pooled4 = consts.tile([128, DC, 2], f32)
        pooled_sb = consts.tile([128, DC], f32)
        pooled_bf = consts.tile([128, DC], bf16)
        hidg_sb = consts.tile([128, 2], f32)
        g_sb = consts.tile([1, 1], f32)
        gb_sb = consts.tile([128, 1], f32)
        bias_sb = consts.tile([128, NJ, 4], f32)
        bias_dve = consts.tile([128, NJ, 4], f32)

        # pooled partial sums (all on DVE; it is idle pre-stream)
        for jc2 in range(2):
            for dc in range(DC):
                nc.vector.reduce_sum(pooled4[:, dc, jc2:jc2 + 1],
                                     xts[:, jc2, dc, :],
                                     axis=mybir.AxisListType.X)
        for dc in range(DC):
            nc.vector.reduce_sum(pooled_sb[:, dc:dc + 1], pooled4[:, dc, :],
                                 axis=mybir.AxisListType.X)
        nc.vector.tensor_copy(pooled_bf, pooled_sb)

        qt_sb = qkv.tile([128, 2, n], bf16)
        ktp_sb = qkv.tile([128, 2, n], bf16)
        vaug_sb = qkv.tile([128, NJ, 4, 65], bf16)
        nc.vector.memset(vaug_sb, 1.0)
        hidc_sb = big2.tile([128, 2, n], bf16, tag="big", name="hidc")
        attn_sb = big2.tile([128, 2, n], bf16, tag="big", name="attn")

        # ================= stream with interleaved jobs =================
        with tc.tile_pool(name="pss", bufs=3, space="PSUM") as pss, \
             tc.tile_pool(name="pop", bufs=3, space="PSUM") as pop, \
             tc.tile_pool(name="pj", bufs=2, space="PSUM") as pj:

            cp_i = [0]

            def cp_eng():
                cp_i[0] += 1
                return nc.vector if cp_i[0] % 2 == 0 else nc.scalar

            def copy(eng, dst, src):
                if eng is nc.scalar:
                    eng.copy(dst, src)
                else:
                    eng.tensor_copy(dst, src)

            def qk_job(cb, ic):
                pq = pj.tile([128, 512], f32, tag="job", name=f"pq{cb}_{ic}")
                jc2, sub = ic // 2, (ic % 2) * 512
                for dc in range(DC):
                    nc.tensor.matmul(pq, wqk_sb[:, cb, dc, :],
                                     xts[:, jc2, dc, sub:sub + 512],
                                     start=(dc == 0), stop=(dc == DC - 1))
                dst = (ktp_sb[:, cb - 2, ic * 512:(ic + 1) * 512] if cb >= 2
                       else qt_sb[:, cb, ic * 512:(ic + 1) * 512])
                copy(cp_eng(), dst, pq)

            def v_job(it):
                pv = pj.tile([128, 4, 64], f32, tag="job", name=f"pv{it}")
                jc2, col = it // 8, (it % 8) * 128
                for dc in range(DC):
                    nc.tensor.matmul(pv, xts[:, jc2, dc, col:col + 128],
                                     wv_sb[:, dc, :],
                                     start=(dc == 0), stop=(dc == DC - 1))
                nc.scalar.copy(vaug_sb[:, it, :, 0:64], pv)

            def covh_job(mc, q):
                ph = pj.tile([128, 512], f32, tag="job", name=f"ph{mc}_{q}")
                nc.tensor.matmul(ph, wce1_sb[:, mc * 128:(mc + 1) * 128],
                                 covT_sb[:, q * 512:(q + 1) * 512],
                                 start=True, stop=True)
                nc.scalar.activation(
                    out=hidc_sb[:, mc, q * 512:(q + 1) * 512],
                    in_=ph, func=AFT.Silu, bias=bce1_sb[:, mc:mc + 1], scale=1.0)

            def covb_job(jt4):
                pc = pj.tile([128, 512], f32, tag="job", name=f"pc{jt4}")
                for k in range(4):
                    jt = jt4 * 4 + k
                    for mc in range(2):
                        nc.tensor.matmul(pc[:, k * 4:k * 4 + 4],
                                         hidc_sb[:, mc, jt * 128:(jt + 1) * 128],
                                         wce2_sb[:, mc * 4:(mc + 1) * 4],
                                         start=(mc == 0), stop=(mc == 1))
                for k in range(4):
                    jt = jt4 * 4 + k
                    nc.vector.tensor_add(bias_sb[:, jt, :], pc[:, k * 4:k * 4 + 4],
                                         bce2_sb)

            def gate_job():
                pg = pj.tile([128, 512], f32, tag="job", name="pg")
                for mc in range(2):
                    for dc in range(DC):
                        nc.tensor.matmul(pg[:, mc:mc + 1],
                                         wfg1_sb[:, dc, mc * 128:(mc + 1) * 128],
                                         pooled_bf[:, dc:dc + 1],
                                         start=(dc == 0), stop=(dc == DC - 1))
                for mc in range(2):
                    nc.scalar.activation(out=hidg_sb[:, mc:mc + 1],
                                         in_=pg[:, mc:mc + 1], func=AFT.Silu,
                                         bias=bfg1_sb[:, mc:mc + 1], scale=1.0 / n)
                pgp = pj.tile([128, 512], f32, tag="job", name="pgp")
                for mc in range(2):
                    nc.tensor.matmul(pgp[0:1, 0:1], hidg_sb[:, mc:mc + 1],
                                     wfg2_sb[:, mc:mc + 1],
                                     start=(mc == 0), stop=(mc == 1))
                nc.scalar.activation(out=g_sb, in_=pgp[0:1, 0:1], func=AFT.Sigmoid,
                                     bias=bfg2_sb, scale=1.0)
                pgb = pj.tile([128, 512], f32, tag="job", name="pgb")
                nc.tensor.matmul(pgb[:, 0:1], ones_f, g_sb, start=True, stop=True)
                nc.vector.tensor_copy(gb_sb, pgb[:, 0:1])
                nc.vector.tensor_scalar_mul(out=bias_sb[:, :, :],
                                            in0=bias_sb[:, :, :], scalar1=gb_sb)
                nc.vector.tensor_scalar(out=bias_dve[:, :, :], in0=bias_sb[:, :, :],
                                        scalar1=SCHRAUD_A, scalar2=SCHRAUD_B,
                                        op0=ALU.mult, op1=ALU.add)
                # exp table warmup
                warm = consts.tile([1, 128], bf16)
                nc.scalar.activation(out=warm, in_=pgb[0:1, 0:128], func=AFT.Exp,
                                     scale=0.001)

            def oproj_job(it, half):
                py = pj.tile([128, 512], f32, tag="job", name=f"py{it}_{half}")
                for pt in range(2):
                    nc.tensor.matmul(py, attn_sb[:, pt, it * 128:(it + 1) * 128],
                                     wo_sb[:, pt, half * 512:(half + 1) * 512],
                                     start=(pt == 0), stop=(pt == 1))
                y_sb = yp.tile([128, 512], f32, tag="y", name=f"y{it}_{half}")
                nc.scalar.copy(y_sb, py)
                nc.sync.dma_start(out=OUT[it * 128:(it + 1) * 128,
                                          half * 512:(half + 1) * 512], in_=y_sb)

            # Jobs popped DURING the stream (emitted before S(jt+1)/AV(jt)).
            # Ordering rule: every producer must be EMITTED before its
            # first consumer (program-order read-before-write is a race):
            # v_job(it) before AV(jt=it) of block 0, K0 chunk ic before
            # S(4*ic) of block 0, Q0ic1 before block 1, etc. Pops are
            # paired early so v/K stay ahead of the consuming iteration.
            jobs = []
            jobs += [lambda it=it: v_job(it) for it in range(0, 4)]     # 2 dbl-pops
            jobs.append(lambda: v_job(4))
            jobs.append(lambda: qk_job(2, 1))
            jobs.append(lambda: v_job(5))
            jobs.append(lambda: qk_job(2, 2))
            jobs.append(lambda: v_job(6))
            jobs.append(lambda: qk_job(2, 3))
            jobs += [lambda it=it: v_job(it) for it in range(7, 16)]
            jobs.append(lambda: qk_job(0, 1))
            jobs.append(lambda: qk_job(3, 0))
            # block 1 onward
            jobs.append(lambda: qk_job(3, 1))
            jobs.append(lambda: qk_job(3, 2))
            jobs.append(lambda: qk_job(3, 3))
            jobs.append(lambda: qk_job(0, 2))
            jobs.append(lambda: qk_job(1, 0))
            jobs.append(lambda: qk_job(0, 3))
            jobs += [lambda ic=ic: qk_job(1, ic) for ic in range(1, 4)]
            jobs.reverse()   # pop() from the end

            def s_tiles(p, ic, jt):
                out = []
                js = slice(jt * 128, (jt + 1) * 128)
                for hh in range(2):
                    lo = hh * 64
                    ps_ = pss.tile([128, IC_W], f32, tag="s",
                                   name=f"s{p}_{ic}_{jt}_{hh}")
                    nc.tensor.matmul(ps_, ktp_sb[lo:lo + 64, p, js],
                                     qt_sb[lo:lo + 64, p,
                                           ic * IC_W:(ic + 1) * IC_W],
                                     start=True, stop=True)
                    out.append(ps_)
                return out

            # pre-stream: K0 ic0 + Q0 ic0 so block 1 can start, then the
            # work that feeds bias_sb (everything the first exp needs MUST
            # precede the first AV in the in-order PE queue, or it
            # deadlocks behind it), padded with early jobs.
            # pre-stream: the minimum for block 0 + the bias_sb chain
            # (everything the first exp needs MUST precede the first AV
            # in the in-order PE queue, or it deadlocks behind it).
            qk_job(2, 0)
            qk_job(0, 0)
            for q in range(4):
                covh_job(0, q)
                covh_job(1, q)
            for j in range(4):
                covb_job(j)
            gate_job()

            blocks = [(p, ic) for p in range(2) for ic in range(NI)]
            for bi, (p, ic) in enumerate(blocks):
                po = [pop.tile([65, IC_W], f32, tag="o",
                               name=f"po{p}_{ic}_{i}") for i in range(2)]
                pend = s_tiles(p, ic, 0)
                for jt in range(NJ):
                    es = []
                    for hh in range(2):
                        h = 2 * p + hh
                        e = ep.tile([128, IC_W], bf16, tag="e",
                                    name=f"e{p}_{ic}_{jt}_{hh}")
                        if _dve_tile(jt, hh):
                            nc.vector.tensor_scalar(
                                out=e.bitcast(i16), in0=pend[hh],
                                scalar1=SCHRAUD_A * scale,
                                scalar2=bias_dve[:, jt, h:h + 1],
                                op0=ALU.mult, op1=ALU.add)
                        else:
                            nc.scalar.activation(out=e, in_=pend[hh],
                                                 func=AFT.Exp,
                                                 bias=bias_sb[:, jt, h:h + 1],
                                                 scale=scale)
                        es.append(e)
                    # jobs go BEFORE S(jt+1)/AV(jt): producers stay ahead
                    # of their consumers and the PE queue head stays
                    # runnable. Double-pop early in block 0 so v/K jobs
                    # outrun the AV/S that consume them.
                    npop = 2 if (bi == 0 and jt < 8) else 1
                    for _ in range(npop):
                        if jobs:
                            jobs.pop()()
                    if jt + 1 < NJ:
                        pend = s_tiles(p, ic, jt + 1)
                    st, sp = (jt == 0), (jt == NJ - 1)
                    for hh in range(2):
                        h = 2 * p + hh
                        nc.tensor.matmul(po[hh], vaug_sb[:, jt, h, :], es[hh],
                                         start=st, stop=sp)
                # normalize: O^T rows 0..63, denominator row 64
                osl = slice(ic * IC_W, (ic + 1) * IC_W)
                for hh in range(2):
                    lo = hh * 64
                    dd = rp.tile([1, IC_W], f32, tag="dd", name=f"dd{p}_{ic}_{hh}")
                    nc.scalar.copy(dd, po[hh][64:65, :])
                    rr = rp.tile([1, IC_W], f32, tag="rr", name=f"rr{p}_{ic}_{hh}")
                    nc.vector.reciprocal_approx_fast(out=rr, in_=dd)
                    recb = rp.tile([64, IC_W], f32, tag="recb",
                                   name=f"recb{p}_{ic}_{hh}")
                    nc.gpsimd.partition_broadcast(recb, rr)
                    nc.vector.tensor_mul(attn_sb[lo:lo + 64, p, osl],
                                         po[hh][0:64, :], recb)
                if p == 1 and ic < NI - 1:
                    for it in range(ic * 4, ic * 4 + 4):
                        jobs.append(lambda it=it, h=1: oproj_job(it, h))
                        jobs.append(lambda it=it, h=0: oproj_job(it, h))
            while jobs:
                jobs.pop()()

        if dbg:
            nc.sync.dma_start(out=DBG_Q, in_=qt_sb)
            nc.sync.dma_start(out=DBG_K, in_=ktp_sb)
            nc.sync.dma_start(out=DBG_V, in_=vaug_sb)
            nc.sync.dma_start(out=DBG_B, in_=bias_sb)
            nc.sync.dma_start(out=DBG_A, in_=attn_sb)

        # ---- tail: last block's out-projection, DMA direct from PSUM ----
        with tc.tile_pool(name="psy", bufs=2, space="PSUM") as psy:
            for it in range(NJ - 4, NJ):
                py = psy.tile([128, D], f32, tag="y")
                for pt in range(2):
                    for half in range(2):
                        nc.tensor.matmul(
                            py[:, half * 512:(half + 1) * 512],
                            attn_sb[:, pt, it * 128:(it + 1) * 128],
                            wo_sb[:, pt, half * 512:(half + 1) * 512],
                            start=(pt == 0), stop=(pt == 1))
                y_sb = yp.tile([128, D], f32, tag="yt", name=f"ysb{it}")
                if it % 2 == 0:
                    nc.vector.tensor_copy(y_sb, py)
                else:
                    nc.scalar.copy(y_sb, py)
                eng = nc.sync if it % 2 == 0 else nc.scalar
                eng.dma_start(out=OUT[it * 128:(it + 1) * 128, :], in_=y_sb)

    nc.compile()
    return nc


def make_in_maps(x, coverage, w_qkv, w_out, b_out, w_ce1, b_ce1, w_ce2, b_ce2,
                 w_fg1, b_fg1, w_fg2, b_fg2, n=N):
    f = np.float32
    DC = D // 128
    x = np.asarray(x, f)
    coverage = np.asarray(coverage, f)
    w_qkv = np.asarray(w_qkv, f)
    w_out = np.asarray(w_out, f)

    def pmajor(a, inner):
        blocks = a.shape[0] // 128
        return np.ascontiguousarray(
            a.reshape(blocks, 128, inner).transpose(1, 0, 2))

    smf = np.concatenate([
        np.asarray(b_ce1, f).reshape(2, 128).T,
        np.tile(np.asarray(b_ce2, f).reshape(1, 16)[:, 0:4], (128, 1)) * 0,  # per-core below
        np.asarray(b_fg1, f).reshape(2, 128).T,
        np.asarray(w_fg2, f).reshape(2, 128).T,
        np.full((128, 1), np.asarray(b_fg2, f).reshape(()), f),
    ], axis=1)

    in_maps = []
    for c in range(NCORES):
        b, hg = divmod(c, 4)
        cs, ce = hg * 256, (hg + 1) * 256
        wq = w_qkv[:, 0 * D + cs:0 * D + ce]
        wk = w_qkv[:, 1 * D + cs:1 * D + ce]
        wv = w_qkv[:, 2 * D + cs:2 * D + ce]
        wqk4 = np.concatenate([wq, wk], axis=1)
        wqk4 = wqk4.reshape(DC, 128, 4, 128).transpose(1, 2, 0, 3)
        xt = x[b].T.reshape(DC, 128, 2, 1024).transpose(1, 2, 0, 3)
        smf_c = smf.copy()
        smf_c[:, 2:6] = np.tile(
            np.asarray(b_ce2, f)[4 * hg:4 * hg + 4][None, :], (128, 1))
        m = {
            "xT": _bf16(xt),
            "wqk": _bf16(wqk4),
            "wv": _bf16(pmajor(wv, 256)),
            "wo": _bf16(pmajor(w_out[cs:ce, :], D)),
            "cvw": _bf16(np.concatenate([coverage[b, :, 0],
                                         np.asarray(w_ce1, f).reshape(-1)])[None, :]),
            "wce2": _bf16(
                np.asarray(w_ce2, f)[:, 4 * hg:4 * hg + 4].reshape(2, 128, 4)
                .transpose(1, 0, 2).reshape(128, 8)),
            "smf": smf_c,
            "wfg1": _bf16(pmajor(np.asarray(w_fg1, f), 256)),
        }
        in_maps.append(m)
    return in_maps


def kernel(**inputs):
    from concourse.bass_utils import run_bass_kernel_spmd
    if "nc" not in _COMPILED:
        _COMPILED["nc"] = build(N)
    nc = _COMPILED["nc"]
    in_maps = make_in_maps(**inputs)
    res = run_bass_kernel_spmd(nc, in_maps, core_ids=list(range(NCORES)))
    outs = [np.asarray(res.results[c]["out"], dtype=np.float32)
            for c in range(NCORES)]
    b_out = np.asarray(inputs["b_out"], np.float32)
    full = np.stack([
        outs[0] + outs[1] + outs[2] + outs[3] + b_out[None, :],
        outs[4] + outs[5] + outs[6] + outs[7] + b_out[None, :],
    ]).astype(np.float32)
    return full
